# revision 13
# baseline (speedup 1.0000x reference)
"""Trainium2 Bass kernel for nn_DecoderBlock (two chained MHA layers, no out-proj).

Sharding: 8 cores = (batch b = core//2) x (head-half g = core%2).
Each core computes 8 heads (512 feature cols) of self-attention for its batch,
normalizes its half of x1 on device, AllGathers the bf16 x1 halves across the
batch pair, then computes cross-attention for its 8 heads (q2 projection
contracts the full gathered x1 against its own-column slice of wq_cross).

All matmul operands are bf16 (fp32 matmuls run in fp32_mode=HIGH, don't keep
the PE HAM clock-gate warm, and cost extra cycles/row; bf16 runs 1 cyc/row at
2.4 GHz with FWL weight loads). PSUM accumulation stays fp32. Device layout is
feature-major: activations live as [d, s] tiles so every matmul contracts over
the partition dim. Softmax runs on scoresT [sk, sq]: exp on the scalar engine
(scale=1/8 free affine + per-partition src-mask bias), causal upper blocks
skipped, diagonal blocks masked by a 0/1 multiply after exp. The AV matmul
uses lhsT=[v_h | 1] so the softmax denominator falls out as row 64. Phase-1
denominators are DMA-packed into one [8, S] tile, reciprocal'd in a single DVE
call, and broadcast per head with a selector matmul; phase-2 output is
returned unnormalized (+denominator row) and normalized on host.
"""

import sys

if '/opt/trn_rl_repo' not in sys.path:
    sys.path.insert(0, '/opt/trn_rl_repo')

import numpy as np

B, S, D, H, DKH = 4, 1024, 1024, 16, 64
NCORES = 8
HPC = H // 2            # 8 heads per core
CPC = HPC * DKH         # 512 feature cols per core
ST = S // 128           # 8 seq tiles
NDT = D // 128          # 8 feature chunks
AUG = DKH + 1           # 65 (v columns + ones)

_CACHE = {}


def _build_nc():
    import concourse.mybir as mybir
    import concourse.tile as tile
    from concourse import bacc
    from contextlib import ExitStack

    F32 = mybir.dt.float32
    BF16 = mybir.dt.bfloat16
    EXP = mybir.ActivationFunctionType.Exp

    nc = bacc.Bacc("TRN2", target_bir_lowering=False, debug=False,
                   num_devices=NCORES)

    xT_d = nc.declare_dram_parameter("xT", [D, S], BF16, isOutput=False)
    encT_d = nc.declare_dram_parameter("encT", [D, S], BF16, isOutput=False)
    wqsT_d = nc.declare_dram_parameter("wqsT", [D, CPC], BF16, isOutput=False)
    wksT_d = nc.declare_dram_parameter("wksT", [D, CPC], BF16, isOutput=False)
    wvsT_d = nc.declare_dram_parameter("wvsT", [D, CPC], BF16, isOutput=False)
    wqcT_d = nc.declare_dram_parameter("wqcT", [D, CPC], BF16, isOutput=False)
    wkcT_d = nc.declare_dram_parameter("wkcT", [D, CPC], BF16, isOutput=False)
    wvcT_d = nc.declare_dram_parameter("wvcT", [D, CPC], BF16, isOutput=False)
    m01_d = nc.declare_dram_parameter("m01", [ST, 128, 128], BF16, isOutput=False)
    srcb_d = nc.declare_dram_parameter("srcb", [128, ST], F32, isOutput=False)
    esel_d = nc.declare_dram_parameter("esel", [HPC // 2, 4 * DKH], BF16,
                                       isOutput=False)
    out_d = nc.declare_dram_parameter("outT", [HPC * AUG, S], F32, isOutput=True)

    cc_in_a = nc.dram_tensor("cc_in_a", [CPC // 2, S], BF16)
    cc_out_a = nc.dram_tensor("cc_out_a", [CPC, S], BF16)
    cc_in_b = nc.dram_tensor("cc_in_b", [CPC // 2, S], BF16)
    cc_out_b = nc.dram_tensor("cc_out_b", [CPC, S], BF16)
    groups = [[0, 1], [2, 3], [4, 5], [6, 7]]

    def banks(lo, hi):
        res = []
        for b0 in range(0, hi, 512):
            c0, c1 = max(lo, b0), min(hi, b0 + 512)
            if c0 < c1:
                res.append((c0, c1))
        return res

    with tile.TileContext(nc) as tc:
      with nc.allow_low_precision(reason="bf16 matmuls within 2e-2 tolerance"):
        with ExitStack() as stk:
            const = stk.enter_context(tc.tile_pool(name="const", bufs=1))
            xe = stk.enter_context(tc.tile_pool(name="xe", bufs=16))
            wp = stk.enter_context(tc.tile_pool(name="wp", bufs=32))
            qk = stk.enter_context(tc.tile_pool(name="qk", bufs=12))
            vap = stk.enter_context(tc.tile_pool(name="vap", bufs=1))
            atp = stk.enter_context(tc.tile_pool(name="atp", bufs=3))
            o1p = stk.enter_context(tc.tile_pool(name="o1p", bufs=8))
            o2p = stk.enter_context(tc.tile_pool(name="o2p", bufs=3))
            dnp = stk.enter_context(tc.tile_pool(name="dnp", bufs=1))
            x1p = stk.enter_context(tc.tile_pool(name="x1p", bufs=8))
            mmps = stk.enter_context(tc.tile_pool(name="mmps", bufs=3, space="PSUM"))
            avps = stk.enter_context(tc.tile_pool(name="avps", bufs=1, space="PSUM"))

            # ---- load x^T + phase-1 weights first (two HWDGE queues in parallel) ----
            xt = []
            for j in range(NDT):
                t = xe.tile([128, S], BF16, name=f"xt{j}", tag="xe")
                nc.sync.dma_start(out=t[:], in_=xT_d[128 * j:128 * (j + 1), :])
                xt.append(t)

            def load_w(w_d, label, eng=None):
                eng = eng or nc.scalar
                ts = []
                for j in range(NDT):
                    t = wp.tile([128, CPC], BF16, name=f"w{label}{j}", tag="w")
                    eng.dma_start(out=t[:], in_=w_d[128 * j:128 * (j + 1), :])
                    ts.append(t)
                return ts

            wqs = load_w(wqsT_d, "qs")
            wks = load_w(wksT_d, "ks")
            wvs = load_w(wvsT_d, "vs")

            # ---- constants ----
            m01t = []
            for i in range(ST):
                t = const.tile([128, 128], BF16, name=f"m01_{i}")
                nc.sync.dma_start(out=t[:], in_=m01_d[i, :, :])
                m01t.append(t)
            srcb = const.tile([128, ST], F32, name="srcb")
            nc.sync.dma_start(out=srcb[:], in_=srcb_d[:])
            esel = const.tile([HPC // 2, 4 * DKH], BF16, name="esel")
            nc.sync.dma_start(out=esel[:], in_=esel_d[:])
            ones_r8 = const.tile([128, HPC], BF16, name="ones_r8")
            nc.any.memset(ones_r8[:], 1.0)

            # ---- projection helpers ----
            def proj_ct(dst_tile, w_tiles, rhs, label, ct, jorder=None):
                # dst [128, S] bf16 (transposed layout), contraction over NDT chunks
                jorder = jorder or range(NDT)
                ps = mmps.tile([128, S], F32, name=f"ps{label}{ct}", tag="mm")
                for n, j in enumerate(jorder):
                    lhs = w_tiles[j][:, 128 * ct:128 * (ct + 1)]
                    for (c0, c1) in banks(0, S):
                        nc.tensor.matmul(ps[:, c0:c1], lhs, rhs[j][:, c0:c1],
                                         start=(n == 0), stop=(n == NDT - 1))
                nc.vector.tensor_copy(dst_tile[:], ps[:])

            def proj_v(va_tiles, wv_tiles, rhs, label, sts=None):
                for st_ in (sts if sts is not None else range(ST)):
                    ps = mmps.tile([128, S], F32, name=f"psv{label}{st_}", tag="mm")
                    for j in range(NDT):
                        nc.tensor.matmul(ps[:, 0:CPC],
                                         rhs[j][:, 128 * st_:128 * (st_ + 1)],
                                         wv_tiles[j][:, :],
                                         start=(j == 0), stop=(j == NDT - 1))
                    dst3 = va_tiles[st_][:, :].rearrange("p (h a) -> p h a", a=AUG)
                    nc.vector.tensor_copy(dst3[:, :, 0:DKH],
                                          ps[:, 0:CPC].rearrange("p (h d) -> p h d", d=DKH))
                    nc.vector.tensor_copy(dst3[:, :, DKH:AUG],
                                          ones_r8[:, :].rearrange("p (h o) -> p h o", o=1))

            # ---- attention (shared between phases) ----
            def attention(h, q_tiles, k_tiles, va_tiles, out_cb, causal, label):
                th, ro = h // 2, 64 * (h % 2)
                qh = q_tiles[th][ro:ro + DKH, :]
                kh = k_tiles[th][ro:ro + DKH, :]
                avp_t = avps.tile([AUG, S], F32, name=f"av{label}{h}", tag="av")

                def emit_av(i, at, lo):
                    for (c0, c1) in banks(lo, S):
                        stop = (i == ST - 1) if not causal else (
                            i == min(ST - 1, (c1 - 1) // 128))
                        nc.tensor.matmul(avp_t[:, c0:c1],
                                         va_tiles[i][:, AUG * h:AUG * h + AUG],
                                         at[:, c0:c1],
                                         start=(i == 0), stop=stop)

                prev = None
                for i in range(ST):
                    lo = 128 * i if causal else 0
                    scp = mmps.tile([128, S], F32, name=f"sc{label}{h}_{i}", tag="mm")
                    for (c0, c1) in banks(lo, S):
                        nc.tensor.matmul(scp[:, c0:c1],
                                         kh[:, 128 * i:128 * (i + 1)],
                                         qh[:, c0:c1], start=True, stop=True)
                    at = atp.tile([128, S], BF16, name=f"at{label}{h}_{i}", tag="at")
                    if causal:
                        nc.scalar.activation(at[:, lo:S], scp[:, lo:S], EXP, scale=0.125)
                        nc.vector.tensor_mul(at[:, lo:lo + 128], at[:, lo:lo + 128],
                                             m01t[i][:, :])
                    else:
                        nc.scalar.activation(at[:, :], scp[:, :], EXP,
                                             bias=srcb[:, i:i + 1], scale=0.125)
                    if prev is not None:
                        emit_av(*prev)
                    prev = (i, at, lo)
                emit_av(*prev)
                out_cb(h, avp_t)

            # ---- phase 1: q/k proj interleaved with attention head pairs ----
            qt = [qk.tile([128, S], BF16, name=f"qt{ct}", tag="qk") for ct in range(4)]
            kt = [qk.tile([128, S], BF16, name=f"kt{ct}", tag="qk") for ct in range(4)]
            va = [vap.tile([128, HPC * AUG], BF16, name=f"va{st_}", tag=f"va{st_}")
                  for st_ in range(ST)]
            x1t = [x1p.tile([128, S], BF16, name=f"x1t{ct}", tag="x1") for ct in range(4)]
            # denominators packed per head-half so each half's reciprocal +
            # broadcast can run as soon as its 4 heads finish
            denph = [dnp.tile([HPC // 2, S], BF16, name=f"denp{x}", tag=f"denp{x}")
                     for x in range(2)]
            o1s = [None] * HPC

            def self_out(h, avp_t):
                o1 = o1p.tile([AUG, S], BF16, name=f"o1_{h}", tag="o1")
                nc.vector.tensor_copy(o1[:], avp_t[:])
                # pack this head's softmax denominator row (sb2sb DMA does the
                # cross-partition move the DVE can't)
                nc.sync.dma_start(out=denph[h // 4][h % 4:h % 4 + 1, :],
                                  in_=o1[DKH:AUG, :])
                o1s[h] = o1

            def recip_half(x):
                rcp_t = dnp.tile([HPC // 2, S], BF16, name=f"rcp{x}", tag=f"rcp{x}")
                nc.vector.reciprocal(rcp_t[:], denph[x][:])
                return rcp_t

            def norm_head(h, rcp_t):
                th, ro = h // 2, 64 * (h % 2)
                bc = mmps.tile([128, S], F32, name=f"bc{h}", tag="mm")
                for (c0, c1) in banks(0, S):
                    nc.tensor.matmul(bc[0:DKH, c0:c1],
                                     esel[:, DKH * (h % 4):DKH * (h % 4 + 1)],
                                     rcp_t[:, c0:c1], start=True, stop=True)
                nc.vector.tensor_mul(x1t[th][ro:ro + DKH, :], o1s[h][0:DKH, :],
                                     bc[0:DKH, :])

            def exchange_half(x, cc_in, cc_out):
                # ship x1t[2x:2x+2] (4 heads), gather both cores' copies
                for t_ in range(2):
                    nc.sync.dma_start(out=cc_in[128 * t_:128 * (t_ + 1), :],
                                      in_=x1t[2 * x + t_][:])
                nc.gpsimd.collective_compute(
                    "AllGather", mybir.AluOpType.bypass,
                    ins=[cc_in[:]], outs=[cc_out[:]], replica_groups=groups)

            def gather_half(x, cc_out, x1g):
                # cc_out rows: [own-parity-0 feats, own-parity-1 feats] in global
                # feature order -> global j chunks {2x, 2x+1, 2x+4, 2x+5}.
                # SWDGE (gpsimd queue): these loads wait on the collective, and
                # on the gpsimd FIFO they sit right behind its trigger instead
                # of blocking the sync HWDGE queue for later small DMAs.
                for r, j in enumerate((2 * x, 2 * x + 1, 2 * x + 4, 2 * x + 5)):
                    t = x1p.tile([128, S], BF16, name=f"x1g{j}", tag="x1")
                    nc.gpsimd.dma_start(out=t[:], in_=cc_out[128 * r:128 * (r + 1), :])
                    x1g[j] = t

            # enc + phase-2 weights prefetch (DMAs run as slots free up)
            enct = []
            for j in range(NDT):
                t = xe.tile([128, S], BF16, name=f"enct{j}", tag="xe")
                nc.sync.dma_start(out=t[:], in_=encT_d[128 * j:128 * (j + 1), :])
                enct.append(t)
            wkc = load_w(wkcT_d, "kc")
            wvc = load_w(wvcT_d, "vc", eng=nc.sync)
            wqc = load_w(wqcT_d, "qc")
            k2t = [qk.tile([128, S], BF16, name=f"k2t{ct}", tag="qk") for ct in range(4)]
            va2 = [vap.tile([128, HPC * AUG], BF16, name=f"va2_{st_}", tag=f"va{st_}")
                   for st_ in range(ST)]
            x1g = [None] * NDT

            proj_ct(qt[0], wqs, xt, "q", 0)
            proj_ct(kt[0], wks, xt, "k", 0)
            proj_v(va, wvs, xt, "s")
            # k2 projection chunks interleave between attention heads so the PE
            # always has dense independent work while ScalarE runs exp (keeps
            # the HAM clock-gate warm)
            rcp_a = rcp_b = None
            for pair in range(4):
                if pair + 1 < 4:
                    proj_ct(qt[pair + 1], wqs, xt, "q", pair + 1)
                    proj_ct(kt[pair + 1], wks, xt, "k", pair + 1)
                attention(2 * pair, qt, kt, va, self_out, causal=True, label="s")
                proj_ct(k2t[pair], wkc, enct, "k2", pair)
                attention(2 * pair + 1, qt, kt, va, self_out, causal=True, label="s")
                if pair == 1:
                    # heads 0-3 done: reciprocal runs on DVE during pair 2
                    rcp_a = recip_half(0)
                if pair == 2:
                    # broadcast+normalize+exchange of the first half hides under
                    # pair 3
                    for h in range(4):
                        norm_head(h, rcp_a)
                    exchange_half(0, cc_in_a, cc_out_a)
                    gather_half(0, cc_out_a, x1g)

            # second half: v2-proj chunks cover the reciprocal and AllGather
            proj_v(va2, wvc, enct, "c", sts=(0, 1))
            rcp_b = recip_half(1)
            for h in range(4, 8):
                norm_head(h, rcp_b)
            exchange_half(1, cc_in_b, cc_out_b)
            proj_v(va2, wvc, enct, "c", sts=(2, 3, 4, 5, 6, 7))
            gather_half(1, cc_out_b, x1g)

            # ---- q2 projection from the gathered full x1 (a-half chunks first) ----
            q2t = [qk.tile([128, S], BF16, name=f"q2t{ct}", tag="qk") for ct in range(4)]
            for ct in range(4):
                proj_ct(q2t[ct], wqc, x1g, "q2", ct, jorder=(0, 1, 4, 5, 2, 3, 6, 7))

            # ---- phase 2 attention (no mask) ----
            def cross_out(h, avp_t):
                o2 = o2p.tile([AUG, S], F32, name=f"o2_{h}", tag="o2")
                nc.vector.tensor_copy(o2[:], avp_t[:])
                nc.sync.dma_start(out=out_d[AUG * h:AUG * (h + 1), :], in_=o2[:])

            for h in range(HPC):
                attention(h, q2t, k2t, va2, cross_out, causal=False, label="c")

    nc.compile()
    return nc


def _get_nc():
    if 'nc' not in _CACHE:
        _CACHE['nc'] = _build_nc()
    return _CACHE['nc']


def kernel(x, encoder_output, src_mask, tgt_mask,
           wq_self, wk_self, wv_self, wq_cross, wk_cross, wv_cross):
    import os
    import ml_dtypes
    from concourse.bass_utils import run_bass_kernel_spmd

    BF = ml_dtypes.bfloat16
    x = np.asarray(x, np.float32)
    enc = np.asarray(encoder_output, np.float32)
    srcm = np.asarray(src_mask)
    tgtm = np.asarray(tgt_mask)

    # host-side mask conversion
    t2 = tgtm[0, 0]  # [S, S]
    m01 = np.empty((ST, 128, 128), BF)
    for i in range(ST):
        blk = t2[128 * i:128 * (i + 1), 128 * i:128 * (i + 1)]
        m01[i] = (blk != 0).T.astype(BF)  # [sk, sq] orientation
    sv = srcm[0, 0, 0, :]  # [S]
    srcb = np.where(sv == 0, np.float32(-1e9), np.float32(0.0))
    srcb = np.ascontiguousarray(srcb.reshape(ST, 128).T)  # [128, ST]

    # per-head selector for the reciprocal broadcast matmul (h mod 4)
    esel = np.zeros((HPC // 2, 4 * DKH), BF)
    for h in range(4):
        esel[h, DKH * h:DKH * (h + 1)] = 1

    def wT(w, cols):
        return np.ascontiguousarray(np.asarray(w, np.float32)[cols, :].T).astype(BF)

    in_maps = []
    for c in range(NCORES):
        b, g = divmod(c, 2)
        cols = slice(CPC * g, CPC * (g + 1))
        in_maps.append({
            "xT": np.ascontiguousarray(x[b].T).astype(BF),
            "encT": np.ascontiguousarray(enc[b].T).astype(BF),
            "wqsT": wT(wq_self, cols),
            "wksT": wT(wk_self, cols),
            "wvsT": wT(wv_self, cols),
            "wqcT": wT(wq_cross, cols),
            "wkcT": wT(wk_cross, cols),
            "wvcT": wT(wv_cross, cols),
            "m01": m01,
            "srcb": srcb,
            "esel": esel,
        })

    nc = _get_nc()
    trace = bool(int(os.environ.get("KERNEL_TRACE", "0")))
    res = run_bass_kernel_spmd(nc, in_maps, list(range(NCORES)), trace=trace)
    if trace:
        _CACHE['exec_time_ns'] = res.exec_time_ns
        _CACHE['mean_exec_time_ns'] = res.mean_exec_time_ns
        _CACHE['res'] = res

    out = np.empty((B, S, D), np.float32)
    for c in range(NCORES):
        b, g = divmod(c, 2)
        ot = np.asarray(res.results[c]["outT"], np.float32)  # [HPC*AUG, S]
        a3 = ot.reshape(HPC, AUG, S)
        num = a3[:, :DKH, :]                      # [h, d, s]
        den = a3[:, DKH:AUG, :]                   # [h, 1, s]
        blk = (num / den).transpose(2, 0, 1)      # [s, h, d]
        out[b, :, CPC * g:CPC * (g + 1)] = blk.reshape(S, CPC)
    return out


# revision 20
# speedup vs baseline: 1.0221x; 1.0221x over previous
"""Trainium2 Bass kernel for nn_DecoderBlock (two chained MHA layers, no out-proj).

Sharding: 8 cores = (batch b = core//2) x (head-half g = core%2).
Each core computes 8 heads (512 feature cols) of self-attention for its batch,
normalizes its half of x1 on device, AllGathers the bf16 x1 halves across the
batch pair, then computes cross-attention for its 8 heads (q2 projection
contracts the full gathered x1 against its own-column slice of wq_cross).

All matmul operands are bf16 (fp32 matmuls run in fp32_mode=HIGH, don't keep
the PE HAM clock-gate warm, and cost extra cycles/row; bf16 runs 1 cyc/row at
2.4 GHz with FWL weight loads). PSUM accumulation stays fp32. Device layout is
feature-major: activations live as [d, s] tiles so every matmul contracts over
the partition dim. Softmax runs on scoresT [sk, sq]: exp on the scalar engine
(scale=1/8 free affine + per-partition src-mask bias), causal upper blocks
skipped, diagonal blocks masked by a 0/1 multiply after exp. The AV matmul
uses lhsT=[v_h | 1] so the softmax denominator falls out as row 64. Phase-1
denominators are DMA-packed into one [8, S] tile, reciprocal'd in a single DVE
call, and broadcast per head with a selector matmul; phase-2 output is
returned unnormalized (+denominator row) and normalized on host.
"""

import sys

if '/opt/trn_rl_repo' not in sys.path:
    sys.path.insert(0, '/opt/trn_rl_repo')

import numpy as np

B, S, D, H, DKH = 4, 1024, 1024, 16, 64
NCORES = 8
HPC = H // 2            # 8 heads per core
CPC = HPC * DKH         # 512 feature cols per core
ST = S // 128           # 8 seq tiles
NDT = D // 128          # 8 feature chunks
AUG = DKH + 1           # 65 (v columns + ones)

_CACHE = {}


def _build_nc():
    import concourse.mybir as mybir
    import concourse.tile as tile
    from concourse import bacc
    from contextlib import ExitStack

    F32 = mybir.dt.float32
    BF16 = mybir.dt.bfloat16
    EXP = mybir.ActivationFunctionType.Exp

    nc = bacc.Bacc("TRN2", target_bir_lowering=False, debug=False,
                   num_devices=NCORES)

    xT_d = nc.declare_dram_parameter("xT", [D, S], BF16, isOutput=False)
    encT_d = nc.declare_dram_parameter("encT", [D, S], BF16, isOutput=False)
    wqsT_d = nc.declare_dram_parameter("wqsT", [D, CPC], BF16, isOutput=False)
    wksT_d = nc.declare_dram_parameter("wksT", [D, CPC], BF16, isOutput=False)
    wvsT_d = nc.declare_dram_parameter("wvsT", [D, CPC], BF16, isOutput=False)
    wqcT_d = nc.declare_dram_parameter("wqcT", [D, CPC], BF16, isOutput=False)
    wkcT_d = nc.declare_dram_parameter("wkcT", [D, CPC], BF16, isOutput=False)
    wvcT_d = nc.declare_dram_parameter("wvcT", [D, CPC], BF16, isOutput=False)
    ident_d = nc.declare_dram_parameter("ident", [128, 128], BF16, isOutput=False)
    gmask_d = nc.declare_dram_parameter("gmask", [128, 128], BF16, isOutput=False)
    srcb_d = nc.declare_dram_parameter("srcb", [128, ST], F32, isOutput=False)
    esel_d = nc.declare_dram_parameter("esel", [HPC // 2, 4 * DKH], BF16,
                                       isOutput=False)
    out_d = nc.declare_dram_parameter("outT", [HPC * AUG, S], F32, isOutput=True)

    cc_in_a = nc.dram_tensor("cc_in_a", [CPC // 2, S], BF16)
    cc_out_a = nc.dram_tensor("cc_out_a", [CPC, S], BF16)
    cc_in_b = nc.dram_tensor("cc_in_b", [CPC // 2, S], BF16)
    cc_out_b = nc.dram_tensor("cc_out_b", [CPC, S], BF16)
    groups = [[0, 1], [2, 3], [4, 5], [6, 7]]

    def banks(lo, hi):
        res = []
        for b0 in range(0, hi, 512):
            c0, c1 = max(lo, b0), min(hi, b0 + 512)
            if c0 < c1:
                res.append((c0, c1))
        return res

    with tile.TileContext(nc) as tc:
      with nc.allow_low_precision(reason="bf16 matmuls within 2e-2 tolerance"):
        with ExitStack() as stk:
            const = stk.enter_context(tc.tile_pool(name="const", bufs=1))
            xe = stk.enter_context(tc.tile_pool(name="xe", bufs=16))
            wp = stk.enter_context(tc.tile_pool(name="wp", bufs=32))
            qk = stk.enter_context(tc.tile_pool(name="qk", bufs=12))
            vap = stk.enter_context(tc.tile_pool(name="vap", bufs=1))
            atp = stk.enter_context(tc.tile_pool(name="atp", bufs=3))
            o1p = stk.enter_context(tc.tile_pool(name="o1p", bufs=8))
            o2p = stk.enter_context(tc.tile_pool(name="o2p", bufs=3))
            dnp = stk.enter_context(tc.tile_pool(name="dnp", bufs=1))
            x1p = stk.enter_context(tc.tile_pool(name="x1p", bufs=8))
            mmps = stk.enter_context(tc.tile_pool(name="mmps", bufs=3, space="PSUM"))
            avps = stk.enter_context(tc.tile_pool(name="avps", bufs=1, space="PSUM"))

            # ---- load x^T + phase-1 weights first (two HWDGE queues in parallel) ----
            xt = []
            for j in range(NDT):
                t = xe.tile([128, S], BF16, name=f"xt{j}", tag="xe")
                nc.sync.dma_start(out=t[:], in_=xT_d[128 * j:128 * (j + 1), :])
                xt.append(t)

            def load_w(w_d, label, eng=None):
                eng = eng or nc.scalar
                ts = []
                for j in range(NDT):
                    t = wp.tile([128, CPC], BF16, name=f"w{label}{j}", tag="w")
                    eng.dma_start(out=t[:], in_=w_d[128 * j:128 * (j + 1), :])
                    ts.append(t)
                return ts

            wqs = load_w(wqsT_d, "qs")
            wks = load_w(wksT_d, "ks")
            wvs = load_w(wvsT_d, "vs")

            # ---- constants ----
            ident = const.tile([128, 128], BF16, name="ident")
            nc.sync.dma_start(out=ident[:], in_=ident_d[:])
            gmask = const.tile([128, 128], BF16, name="gmask")
            nc.sync.dma_start(out=gmask[:], in_=gmask_d[:])
            srcb = const.tile([128, ST], F32, name="srcb")
            nc.sync.dma_start(out=srcb[:], in_=srcb_d[:])
            esel = const.tile([HPC // 2, 4 * DKH], BF16, name="esel")
            nc.sync.dma_start(out=esel[:], in_=esel_d[:])
            ones_r8 = const.tile([128, HPC], BF16, name="ones_r8")
            nc.any.memset(ones_r8[:], 1.0)

            # ---- projection helpers ----
            def proj_ct(dst_tile, w_tiles, rhs, label, ct, jorder=None):
                # dst [128, S] bf16 (transposed layout), contraction over NDT chunks
                jorder = jorder or range(NDT)
                ps = mmps.tile([128, S], F32, name=f"ps{label}{ct}", tag="mm")
                for n, j in enumerate(jorder):
                    lhs = w_tiles[j][:, 128 * ct:128 * (ct + 1)]
                    for (c0, c1) in banks(0, S):
                        nc.tensor.matmul(ps[:, c0:c1], lhs, rhs[j][:, c0:c1],
                                         start=(n == 0), stop=(n == NDT - 1))
                nc.vector.tensor_copy(dst_tile[:], ps[:])

            def proj_v(va_tiles, wv_tiles, rhs, label, sts=None):
                for st_ in (sts if sts is not None else range(ST)):
                    ps = mmps.tile([128, S], F32, name=f"psv{label}{st_}", tag="mm")
                    for j in range(NDT):
                        nc.tensor.matmul(ps[:, 0:CPC],
                                         rhs[j][:, 128 * st_:128 * (st_ + 1)],
                                         wv_tiles[j][:, :],
                                         start=(j == 0), stop=(j == NDT - 1))
                    dst3 = va_tiles[st_][:, :].rearrange("p (h a) -> p h a", a=AUG)
                    nc.vector.tensor_copy(dst3[:, :, 0:DKH],
                                          ps[:, 0:CPC].rearrange("p (h d) -> p h d", d=DKH))
                    nc.vector.tensor_copy(dst3[:, :, DKH:AUG],
                                          ones_r8[:, :].rearrange("p (h o) -> p h o", o=1))

            # ---- attention (shared between phases) ----
            def attention(h, q_tiles, k_tiles, va_tiles, out_cb, causal, label):
                th, ro = h // 2, 64 * (h % 2)
                qh = q_tiles[th][ro:ro + DKH, :]
                kh = k_tiles[th][ro:ro + DKH, :]
                avp_t = avps.tile([AUG, S], F32, name=f"av{label}{h}", tag="av")

                def emit_av(i, at, lo):
                    for (c0, c1) in banks(lo, S):
                        stop = (i == ST - 1) if not causal else (
                            i == min(ST - 1, (c1 - 1) // 128))
                        nc.tensor.matmul(avp_t[:, c0:c1],
                                         va_tiles[i][:, AUG * h:AUG * h + AUG],
                                         at[:, c0:c1],
                                         start=(i == 0), stop=stop)

                prev = None
                for i in range(ST):
                    lo = 128 * i if causal else 0
                    scp = mmps.tile([128, S], F32, name=f"sc{label}{h}_{i}", tag="mm")
                    for (c0, c1) in banks(lo, S):
                        diag = causal and c0 <= lo < c1
                        nc.tensor.matmul(scp[:, c0:c1],
                                         kh[:, 128 * i:128 * (i + 1)],
                                         qh[:, c0:c1], start=True, stop=not diag)
                        if diag:
                            # causal mask: accumulate -1e9 upper-triangle into the
                            # diagonal block on the PE (keeps DVE off the exp->AV
                            # critical path)
                            nc.tensor.matmul(scp[:, lo:lo + 128], ident[:, :],
                                             gmask[:, :], start=False, stop=True)
                    at = atp.tile([128, S], BF16, name=f"at{label}{h}_{i}", tag="at")
                    if causal:
                        nc.scalar.activation(at[:, lo:S], scp[:, lo:S], EXP, scale=0.125)
                    else:
                        nc.scalar.activation(at[:, :], scp[:, :], EXP,
                                             bias=srcb[:, i:i + 1], scale=0.125)
                    if prev is not None:
                        emit_av(*prev)
                    prev = (i, at, lo)
                emit_av(*prev)
                out_cb(h, avp_t)

            # ---- phase 1: q/k proj interleaved with attention head pairs ----
            qt = [qk.tile([128, S], BF16, name=f"qt{ct}", tag="qk") for ct in range(4)]
            kt = [qk.tile([128, S], BF16, name=f"kt{ct}", tag="qk") for ct in range(4)]
            va = [vap.tile([128, HPC * AUG], BF16, name=f"va{st_}", tag=f"va{st_}")
                  for st_ in range(ST)]
            x1t = [x1p.tile([128, S], BF16, name=f"x1t{ct}", tag="x1") for ct in range(4)]
            # denominators packed per head-half so each half's reciprocal +
            # broadcast can run as soon as its 4 heads finish
            denph = [dnp.tile([HPC // 2, S], BF16, name=f"denp{x}", tag=f"denp{x}")
                     for x in range(2)]
            o1s = [None] * HPC

            def self_out(h, avp_t):
                o1 = o1p.tile([AUG, S], BF16, name=f"o1_{h}", tag="o1")
                nc.vector.tensor_copy(o1[:], avp_t[:])
                # pack this head's softmax denominator row (sb2sb DMA does the
                # cross-partition move the DVE can't)
                nc.sync.dma_start(out=denph[h // 4][h % 4:h % 4 + 1, :],
                                  in_=o1[DKH:AUG, :])
                o1s[h] = o1

            def recip_half(x):
                rcp_t = dnp.tile([HPC // 2, S], BF16, name=f"rcp{x}", tag=f"rcp{x}")
                nc.vector.reciprocal(rcp_t[:], denph[x][:])
                return rcp_t

            def norm_head(h, rcp_t):
                th, ro = h // 2, 64 * (h % 2)
                bc = mmps.tile([128, S], F32, name=f"bc{h}", tag="mm")
                for (c0, c1) in banks(0, S):
                    nc.tensor.matmul(bc[0:DKH, c0:c1],
                                     esel[:, DKH * (h % 4):DKH * (h % 4 + 1)],
                                     rcp_t[:, c0:c1], start=True, stop=True)
                nc.vector.tensor_mul(x1t[th][ro:ro + DKH, :], o1s[h][0:DKH, :],
                                     bc[0:DKH, :])

            def exchange_half(x, cc_in, cc_out):
                # ship x1t[2x:2x+2] (4 heads), gather both cores' copies
                for t_ in range(2):
                    nc.sync.dma_start(out=cc_in[128 * t_:128 * (t_ + 1), :],
                                      in_=x1t[2 * x + t_][:])
                nc.gpsimd.collective_compute(
                    "AllGather", mybir.AluOpType.bypass,
                    ins=[cc_in[:]], outs=[cc_out[:]], replica_groups=groups)

            def gather_half(x, cc_out, x1g):
                # cc_out rows: [own-parity-0 feats, own-parity-1 feats] in global
                # feature order -> global j chunks {2x, 2x+1, 2x+4, 2x+5}.
                # NOTE: these waits block the issuing HWDGE FIFO, so gathers are
                # emitted only after every DMA that must not wait on a collective.
                for r, j in enumerate((2 * x, 2 * x + 1, 2 * x + 4, 2 * x + 5)):
                    t = x1p.tile([128, S], BF16, name=f"x1g{j}", tag="x1")
                    nc.sync.dma_start(out=t[:], in_=cc_out[128 * r:128 * (r + 1), :])
                    x1g[j] = t

            # enc + phase-2 weights prefetch (DMAs run as slots free up)
            enct = []
            for j in range(NDT):
                t = xe.tile([128, S], BF16, name=f"enct{j}", tag="xe")
                nc.sync.dma_start(out=t[:], in_=encT_d[128 * j:128 * (j + 1), :])
                enct.append(t)
            wkc = load_w(wkcT_d, "kc")
            wvc = load_w(wvcT_d, "vc", eng=nc.sync)
            wqc = load_w(wqcT_d, "qc")
            k2t = [qk.tile([128, S], BF16, name=f"k2t{ct}", tag="qk") for ct in range(4)]
            va2 = [vap.tile([128, HPC * AUG], BF16, name=f"va2_{st_}", tag=f"va{st_}")
                   for st_ in range(ST)]
            x1g = [None] * NDT

            proj_ct(qt[0], wqs, xt, "q", 0)
            proj_ct(kt[0], wks, xt, "k", 0)
            proj_v(va, wvs, xt, "s")
            # k2 projection chunks interleave between attention heads so the PE
            # always has dense independent work while ScalarE runs exp (keeps
            # the HAM clock-gate warm)
            rcp_a = rcp_b = None
            for pair in range(4):
                if pair + 1 < 4:
                    proj_ct(qt[pair + 1], wqs, xt, "q", pair + 1)
                    proj_ct(kt[pair + 1], wks, xt, "k", pair + 1)
                attention(2 * pair, qt, kt, va, self_out, causal=True, label="s")
                proj_ct(k2t[pair], wkc, enct, "k2", pair)
                attention(2 * pair + 1, qt, kt, va, self_out, causal=True, label="s")
                if pair == 1:
                    # heads 0-3 done: reciprocal runs on DVE during pair 2
                    rcp_a = recip_half(0)
                if pair == 2:
                    # broadcast+normalize+exchange of the first half hides under
                    # pair 3
                    for h in range(4):
                        norm_head(h, rcp_a)
                    exchange_half(0, cc_in_a, cc_out_a)

            # second half: v2-proj chunks cover the reciprocal and AllGather
            proj_v(va2, wvc, enct, "c", sts=(0, 1))
            rcp_b = recip_half(1)
            for h in range(4, 8):
                norm_head(h, rcp_b)
            exchange_half(1, cc_in_b, cc_out_b)
            gather_half(0, cc_out_a, x1g)
            proj_v(va2, wvc, enct, "c", sts=(2, 3, 4, 5, 6, 7))
            gather_half(1, cc_out_b, x1g)

            # ---- q2 projection from the gathered full x1 (a-half chunks first) ----
            q2t = [qk.tile([128, S], BF16, name=f"q2t{ct}", tag="qk") for ct in range(4)]
            for ct in range(4):
                proj_ct(q2t[ct], wqc, x1g, "q2", ct, jorder=(0, 1, 4, 5, 2, 3, 6, 7))

            # ---- phase 2 attention (no mask) ----
            def cross_out(h, avp_t):
                o2 = o2p.tile([AUG, S], F32, name=f"o2_{h}", tag="o2")
                nc.vector.tensor_copy(o2[:], avp_t[:])
                nc.sync.dma_start(out=out_d[AUG * h:AUG * (h + 1), :], in_=o2[:])

            for h in range(HPC):
                attention(h, q2t, k2t, va2, cross_out, causal=False, label="c")

    nc.compile()
    return nc


def _get_nc():
    if 'nc' not in _CACHE:
        _CACHE['nc'] = _build_nc()
    return _CACHE['nc']


def kernel(x, encoder_output, src_mask, tgt_mask,
           wq_self, wk_self, wv_self, wq_cross, wk_cross, wv_cross):
    import os
    import ml_dtypes
    from concourse.bass_utils import run_bass_kernel_spmd

    BF = ml_dtypes.bfloat16
    x = np.asarray(x, np.float32)
    enc = np.asarray(encoder_output, np.float32)
    srcm = np.asarray(src_mask)
    tgtm = np.asarray(tgt_mask)

    # host-side mask conversion: diagonal-block mask as an additive -1e9
    # upper-triangle (scoresT [sk, sq] orientation), applied on-device via
    # ident^T @ gmask accumulation
    t2 = tgtm[0, 0]  # [S, S]
    blk = (t2[0:128, 0:128] != 0).T  # [sk, sq]; same for every diagonal block
    gmask = np.where(blk, np.float32(0.0), np.float32(-1e9)).astype(BF)
    ident = np.eye(128, dtype=np.float32).astype(BF)
    sv = srcm[0, 0, 0, :]  # [S]
    srcb = np.where(sv == 0, np.float32(-1e9), np.float32(0.0))
    srcb = np.ascontiguousarray(srcb.reshape(ST, 128).T)  # [128, ST]

    # per-head selector for the reciprocal broadcast matmul (h mod 4)
    esel = np.zeros((HPC // 2, 4 * DKH), BF)
    for h in range(4):
        esel[h, DKH * h:DKH * (h + 1)] = 1

    def wT(w, cols):
        return np.ascontiguousarray(np.asarray(w, np.float32)[cols, :].T).astype(BF)

    in_maps = []
    for c in range(NCORES):
        b, g = divmod(c, 2)
        cols = slice(CPC * g, CPC * (g + 1))
        in_maps.append({
            "xT": np.ascontiguousarray(x[b].T).astype(BF),
            "encT": np.ascontiguousarray(enc[b].T).astype(BF),
            "wqsT": wT(wq_self, cols),
            "wksT": wT(wk_self, cols),
            "wvsT": wT(wv_self, cols),
            "wqcT": wT(wq_cross, cols),
            "wkcT": wT(wk_cross, cols),
            "wvcT": wT(wv_cross, cols),
            "ident": ident,
            "gmask": gmask,
            "srcb": srcb,
            "esel": esel,
        })

    nc = _get_nc()
    trace = bool(int(os.environ.get("KERNEL_TRACE", "0")))
    res = run_bass_kernel_spmd(nc, in_maps, list(range(NCORES)), trace=trace)
    if trace:
        _CACHE['exec_time_ns'] = res.exec_time_ns
        _CACHE['mean_exec_time_ns'] = res.mean_exec_time_ns
        _CACHE['res'] = res

    out = np.empty((B, S, D), np.float32)
    for c in range(NCORES):
        b, g = divmod(c, 2)
        ot = np.asarray(res.results[c]["outT"], np.float32)  # [HPC*AUG, S]
        a3 = ot.reshape(HPC, AUG, S)
        num = a3[:, :DKH, :]                      # [h, d, s]
        den = a3[:, DKH:AUG, :]                   # [h, 1, s]
        blk = (num / den).transpose(2, 0, 1)      # [s, h, d]
        out[b, :, CPC * g:CPC * (g + 1)] = blk.reshape(S, CPC)
    return out
